# revision 60
# baseline (speedup 1.0000x reference)
"""Causal multi-head attention (B=4, T=2048, H=1024, 16 heads) on 8 trn2 cores.

Sharding: batch(4) x head-group(2).  Core c -> batch b=c//2, heads g=c%2
(8 heads each): zero-communication data/tensor parallelism.  Each core
computes its QKV projection slice, causal+padding-masked attention for its 8
heads, and a row-parallel slice of the output projection; the two partial
outputs per batch row are summed on the host (row-parallel unshard).

Device algorithm (per core; attention kept transposed so softmax reduces
along the PE contraction dim, all matmuls at 1 cycle/row):
  DMA issue costs ~650ns each on the sync queue, so inputs arrive as a few
  large host-packed contiguous transfers in priority order: the first QK
  psum needs only ~1.25 MB (wqk ct0 + x column-block 0).
  ScalarE's exp stream is the phase-2 bottleneck (~171us irreducible at
  1 col/cycle), so projections are BRAIDED INTO the attention stream: a
  prologue emits only QK t-tile 0 and V chunks 0..3; QK tile tt and V
  chunks for q-tile qt+1 (plus the previous q-tile's output-projection
  tiles) are spread as PE filler across q-tile qt's S-pair slots, starting
  exp ~40us after launch and keeping the PE busy where exp gates PV.
  All-padded key chunks beyond ceil(max_len/128) (derived from the mask at
  runtime; program compiled+cached per value) are skipped entirely in
  S/exp/PV/V, and the K projection stops at that column.
  QT/KT [512, T] bf16 = wqk^T-slices @ xT  (Q pre-scaled by 1/sqrt(hd) on
      host; bias added on DVE during the PSUM->SBUF move)
  V [T, 8x65] bf16 = xT^T @ wv, bias via DVE add of a gpsimd-broadcast
      replicated row; a ones column per head; all 65 columns multiplied by
      the key-padding 0/1 mask (per-partition scalar) -> padded keys drop
      out of both the attention numerator and the softmax denominator.
  per (head, q-tile 512, k-chunk pair 2x128), exact-triangle streaming:
      S^T[k, q] = KT_h[:, chunk].T @ QT_h[:, qtile]     (bf16, f32 PSUM);
      diagonal chunks stream only q >= 128*c (partial-range matmul)
      P^T = exp(S^T)  (ScalarE, ONE op per pair over [s_even:1024) -- the
      never-read gap columns of diagonal pairs hold stale-but-finite psum)
      causal masking only on the 128x128 diagonal block of diagonal chunks:
      P^T *= upper-tri 0/1 mask on DVE (post-exp, off the ScalarE path)
      o^T[65, q] += V_aug[chunk, head].T @ P^T          (row 64 = denom;
      diagonal chunks accumulate only their valid q sub-range)
  softmax tail: copy denom row, DMA-shift to partition 0, approx-recip,
  gpsimd partition-broadcast, scale on DVE.  (partition_broadcast reads
  PHYSICAL partition 0 on hw regardless of the AP base, and the custom-DVE
  recip cannot read PSUM -- both were tried and produce garbage.)
  Odd head of each pair runs FIRST so the pair's last writer is the even
  head's direct [0:64) write -- the final y tiles never wait on a DMA shift.
  y[t, j] = sum_hp o_dense_hp[:, t].T @ wout_hp[:, j] (bf16) + b_out on DVE,
  stored bf16 (host upcasts and sums the two per-batch partials in f32)
"""

import os
import sys

import numpy as np

sys.path.insert(0, "/opt/trn_rl_repo")

B, T, H = 4, 2048, 1024
NH, HD = 16, 64
NCORES = 8
HPC = 8          # heads per core
GD = HPC * HD    # head dims per core = 512
KC = T // 128    # 16 k-chunks
QT_TILES = T // 512  # 4 q-tiles
HC = H // 128    # 8 h-chunks (contraction for projections)


def _build_nc(nkc=KC):
    import concourse.bass as bass
    import concourse.tile as tile
    import concourse.mybir as mybir
    from concourse import bacc
    from contextlib import ExitStack

    f32 = mybir.dt.float32
    bf16 = mybir.dt.bfloat16
    EXP = mybir.ActivationFunctionType.Exp

    nc = bacc.Bacc("TRN2", target_bir_lowering=False, debug=False)

    xp_d = nc.dram_tensor("xp", [128, 4 * 4096], bf16, kind="ExternalInput").ap()
    wqkp_d = nc.dram_tensor("wqkp", [128, 8 * 1024], bf16, kind="ExternalInput").ap()
    wvp_d = nc.dram_tensor("wvp", [128, 8 * 512], bf16, kind="ExternalInput").ap()
    pbq_d = nc.dram_tensor("pbq", [128, KC + 8], f32, kind="ExternalInput").ap()
    bv_d = nc.dram_tensor("bv", [1, GD], f32, kind="ExternalInput").ap()
    woutp_d = nc.dram_tensor("woutp", [128, 4 * H], bf16, kind="ExternalInput").ap()
    bout_d = nc.dram_tensor("bout", [1, H], f32, kind="ExternalInput").ap()
    tri_d = nc.dram_tensor("tri", [128, 128], bf16, kind="ExternalInput").ap()
    y_d = nc.dram_tensor("y", [T, H], bf16, kind="ExternalOutput").ap()

    def emit_v_proj(nc, bvrep, xtb, wv_sb, psv, v_sb, padb01_sb, ts, HC, HPC):
        tt, off = ts // 4, (ts % 4) * 128
        for hc in range(HC):
            nc.tensor.matmul(
                psv, xtb[tt][:, hc * 512 + off:hc * 512 + off + 128],
                wv_sb[:, hc * 512:(hc + 1) * 512],
                start=(hc == 0), stop=(hc == HC - 1))
        pad_c = padb01_sb[:, ts:ts + 1]
        dst = v_sb[ts].rearrange("p (h c) -> p h c", h=HPC)[:, :, 0:64]
        bsrc = bvrep.rearrange("p (h c) -> p h c", h=HPC)
        srcv = psv.rearrange("p (h c) -> p h c", h=HPC)
        nc.vector.tensor_add(dst, bsrc, srcv)
        nc.vector.tensor_scalar_mul(dst, dst, pad_c)
        onescols = v_sb[ts].rearrange("p (h c) -> p h c", h=HPC)[:, :, 64:65]
        nc.vector.memset(onescols, 1.0)
        nc.vector.tensor_scalar_mul(onescols, onescols, pad_c)

    with ExitStack() as ctx:
        tc = ctx.enter_context(tile.TileContext(nc))

        # Persistent activations
        acts = ctx.enter_context(tc.tile_pool(name="acts", bufs=1))
        qk_sb = [acts.tile([128, T], bf16, name=f"qk{i}") for i in range(8)]
        v_sb = [acts.tile([128, HPC * 65], bf16, name=f"v{c}") for c in range(KC)]

        # Inputs, few big DMAs in priority order: pbq, wqk(ct0), x(tt0),
        # wqk(ct1-7), x(tt1-3), wv, bv.
        p1c = ctx.enter_context(tc.tile_pool(name="p1c", bufs=1))
        pbq_sb = p1c.tile([128, KC + 8], f32, name="pbq_sb")
        padb01_sb = pbq_sb[:, 0:KC]
        bqkc_sb = pbq_sb[:, KC:KC + 8]
        nc.sync.dma_start(pbq_sb, pbq_d)

        xt_pool = ctx.enter_context(tc.tile_pool(name="xt", bufs=1))
        xtb = [xt_pool.tile([128, 8 * 512], bf16, name=f"xt{t}") for t in range(4)]
        wqk_pool = ctx.enter_context(tc.tile_pool(name="wqkp", bufs=1))
        wqkp_sb = wqk_pool.tile([128, 8 * 1024], bf16, name="wqkp_sb")
        wv_pool = ctx.enter_context(tc.tile_pool(name="wvp", bufs=1))
        wv_sb = wv_pool.tile([128, 8 * 512], bf16, name="wv_sb")

        nc.sync.dma_start(wqkp_sb[:, 0:1024], wqkp_d[:, 0:1024])
        nc.sync.dma_start(xtb[0][:, 0:2048], xp_d[:, 0:2048])
        nc.sync.dma_start(xtb[0][:, 2048:4096], xp_d[:, 2048:4096])
        nc.sync.dma_start(wqkp_sb[:, 1024:4096], wqkp_d[:, 1024:4096])
        nc.sync.dma_start(wqkp_sb[:, 4096:8192], wqkp_d[:, 4096:8192])
        nc.sync.dma_start(wv_sb, wvp_d)
        bv_sb = p1c.tile([1, GD], f32, name="bv_sb")
        nc.sync.dma_start(bv_sb, bv_d)
        tri_sb = p1c.tile([128, 128], bf16, name="tri_sb")
        nc.sync.dma_start(tri_sb, tri_d)
        for tt in range(1, 4):
            nc.sync.dma_start(xtb[tt], xp_d[:, tt * 4096:(tt + 1) * 4096])
        bvrep = p1c.tile([128, GD], f32, name="bvrep")
        nc.gpsimd.partition_broadcast(bvrep, bv_sb)

        # PE p-state warm-up staging: borrow qk_sb[0] (overwritten later by
        # the real projection) as a memset-zeroed matmul operand so the
        # warm-up has no DMA dependency.
        warm_sb = qk_sb[0][:, 0:640]
        nc.vector.memset(warm_sb, 0.0)

        # ------------- single phase: projections braided into attention ----
        with ExitStack() as p2:
            p2c = p2.enter_context(tc.tile_pool(name="p2c", bufs=1))
            bout_sb = p2c.tile([1, H], f32, name="bout_sb")
            nc.sync.dma_start(bout_sb, bout_d)
            brep = p2c.tile([128, H], f32, name="brep")
            nc.gpsimd.partition_broadcast(brep, bout_sb)
            wout_sb = p2c.tile([128, 4 * H], bf16, name="wo")
            nc.sync.dma_start(wout_sb, woutp_d)

            ppool = p2.enter_context(tc.tile_pool(name="pchunks", bufs=16))
            osc_pool = p2.enter_context(tc.tile_pool(name="osc", bufs=4))
            oden_pool = p2.enter_context(tc.tile_pool(name="oden", bufs=12))
            dpool = p2.enter_context(tc.tile_pool(name="dtiles", bufs=4))
            ypool = p2.enter_context(tc.tile_pool(name="ysb", bufs=3))
            ps_s = p2.enter_context(tc.tile_pool(name="ps_s", bufs=2, space="PSUM"))
            ps_o = p2.enter_context(tc.tile_pool(name="ps_o", bufs=2, space="PSUM"))
            ps_y = p2.enter_context(tc.tile_pool(name="ps_y", bufs=2, space="PSUM"))

            def emit_qk_ct(tt, ct):
                """one Q^T/K^T projection column-tile (K clamped to nkc keys)"""
                w = min(512, max(0, nkc * 128 - tt * 512)) if ct >= 4 else 512
                if w == 0:
                    return
                ps = ps_y.tile([128, 512], f32, tag="y", name=f"psqk{ct}_{tt}")
                for hc in range(HC):
                    nc.tensor.matmul(
                        ps[:, 0:w],
                        wqkp_sb[:, ct * 1024 + hc * 128:ct * 1024 + (hc + 1) * 128],
                        xtb[tt][:, hc * 512:hc * 512 + w],
                        start=(hc == 0), stop=(hc == HC - 1))
                nc.vector.tensor_scalar_add(
                    qk_sb[ct][:, tt * 512:tt * 512 + w], ps[:, 0:w],
                    bqkc_sb[:, ct:ct + 1])

            warm_ps = ps_y.tile([128, 512], f32, tag="y", name="warm_ps")
            for _ in range(16):
                nc.tensor.matmul(warm_ps, warm_sb[:, 0:128], warm_sb[:, 128:640],
                                 start=True, stop=True)

            # Prologue: only the QK tiles and V chunks q-tile 0 needs.  The
            # remaining projection tiles are braided into the attention
            # stream as PE filler -- ScalarE's exp stream (the phase-2
            # bottleneck) starts ~45us earlier this way.
            for ct in range(8):
                emit_qk_ct(0, ct)
            for ts in range(4):
                emit_v_proj(nc, bvrep, xtb, wv_sb,
                            ps_y.tile([128, 512], f32, tag="y", name=f"psv{ts}"),
                            v_sb, padb01_sb, ts, HC, HPC)

            def attn_tail(qt, h, opsum, o_dense):
                """softmax denom -> recip (straight off psum p64) -> DMA shift
                to p0 -> gpsimd broadcast -> scale -> dense repack"""
                stage = dpool.tile([65, 512], f32, tag="dstage", name=f"st{qt}_{h}")
                nc.vector.tensor_copy(stage[64:65, :], opsum[64:65, :])
                dp0 = dpool.tile([1, 512], f32, tag="dp0", name=f"dp0_{qt}_{h}")
                nc.sync.dma_start(dp0, stage[64:65, :])
                rp0 = dpool.tile([1, 512], f32, tag="rp0", name=f"rp0_{qt}_{h}")
                nc.vector.reciprocal_approx_fast(rp0, dp0)
                rrep = dpool.tile([64, 512], f32, tag="rrep", name=f"rr{qt}_{h}")
                if qt == QT_TILES - 1 and h == 6:
                    # last head of the run: halve the broadcast+scale so the
                    # first final-y tiles (reading cols 0:256) start ~1us
                    # sooner -- this chain is the end-of-run critical path
                    for c0 in (0, 256):
                        nc.gpsimd.partition_broadcast(
                            rrep[:, c0:c0 + 256], rp0[:, c0:c0 + 256])
                        nc.vector.tensor_mul(o_dense[0:64, c0:c0 + 256],
                                             rrep[:, c0:c0 + 256],
                                             opsum[0:64, c0:c0 + 256])
                    return
                nc.gpsimd.partition_broadcast(rrep, rp0)
                if h % 2 == 0:
                    nc.vector.tensor_mul(o_dense[0:64, :], rrep, opsum[0:64, :])
                else:
                    o_sc = osc_pool.tile([64, 512], bf16, tag="osc", name=f"osc{qt}_{h}")
                    nc.vector.tensor_mul(o_sc, rrep, opsum[0:64, :])
                    nc.sync.dma_start(o_dense[64:128, :], o_sc)

            def emit_y_tile(qt, j, ts, oden):
                """one output-projection tile for q-tile qt (b_out via preload)"""
                q0 = qt * 512
                ypsum = ps_y.tile([128, 512], f32, tag="y", name=f"y{qt}_{j}_{ts}")
                for hp in range(4):
                    nc.tensor.matmul(
                        ypsum,
                        oden[hp][:, ts * 128:(ts + 1) * 128],
                        wout_sb[:, hp * 1024 + j * 512:hp * 1024 + (j + 1) * 512],
                        start=(hp == 0), stop=(hp == 3))
                ysb = ypool.tile([128, 512], bf16, tag="ysb", name=f"ys{qt}_{j}_{ts}")
                nc.vector.tensor_add(ysb, brep[:, j * 512:(j + 1) * 512], ypsum)
                nc.sync.dma_start(
                    y_d[q0 + ts * 128:q0 + (ts + 1) * 128, j * 512:(j + 1) * 512],
                    ysb)

            # Just-in-time braided PE filler, spread evenly over each q-tile's
            # pair slots.  QK projection tile tt (needed first by q-tile tt)
            # and V chunks arrive one q-tile before first use; y tiles of
            # q-tile 0 braid into qt1, y of 1 and 2 into qt3 (the most
            # exp-bound stretch), y of 3 flush at the end.
            fillers = {qt: [] for qt in range(QT_TILES)}
            for vqt in range(QT_TILES - 1):
                for ct in range(8):
                    fillers[vqt].append(
                        lambda tt=vqt + 1, ct=ct: emit_qk_ct(tt, ct))
                for c in range(4 * (vqt + 1), min(4 * (vqt + 2), nkc)):
                    fillers[vqt].append(
                        lambda c=c: emit_v_proj(
                            nc, bvrep, xtb, wv_sb,
                            ps_y.tile([128, 512], f32, tag="y", name=f"psvd{c}"),
                            v_sb, padb01_sb, c, HC, HPC))

            def emit_pv_tail(h, pts, starts, o_dense, pqt, pnk):
                opsum = ps_o.tile([65, 512], f32, tag="o", name=f"o{pqt}_{h}")
                for c in range(pnk):
                    s = starts[c]
                    nc.tensor.matmul(
                        opsum[:, s:512],
                        v_sb[c][:, h * 65:(h + 1) * 65].bitcast(bf16),
                        pts[c // 2][:, (c % 2) * 512 + s:(c % 2) * 512 + 512],
                        start=(c == 0), stop=(c == pnk - 1),
                        skip_group_check=True)
                attn_tail(pqt, h, opsum, o_dense)

            # one-head software-pipeline skew, carried ACROSS q-tile
            # boundaries: S(next head) always runs between S(h) and PV(h)
            # so exp(h) has a full extra S-phase to drain before PV fires
            pend = []
            late_y = []
            for qt in range(QT_TILES):
                q0 = qt * 512
                nk = min(4 * (qt + 1), nkc)
                nslots = ((nk + 1) // 2) * HPC
                nfill = len(fillers[qt])
                pops = {round((i + 1) * nslots / (nfill + 1)) for i in range(nfill)}
                slot = 0
                oden = []
                for h in [1, 0, 3, 2, 5, 4, 7, 6]:
                    if len(oden) <= h // 2:
                        o_dense = oden_pool.tile([128, 512], bf16, tag="od",
                                                 name=f"od{qt}_{h // 2}")
                        oden.append(o_dense)
                    else:
                        o_dense = oden[h // 2]
                    hq = qk_sb[h // 2][(h % 2) * 64:(h % 2) * 64 + 64, q0:q0 + 512]
                    # S^T in two-chunk psum tiles, whole-head S stream first
                    # (exp trails on ScalarE), then the dense PV stream.
                    # Diagonal chunks stream/accumulate only q >= 128*c.
                    pts = []
                    starts = []
                    for cc in range((nk + 1) // 2):
                        ncc = min(2, nk - 2 * cc)   # chunks in this pair
                        spsum = ps_s.tile([128, 1024], f32, tag="s",
                                          name=f"s{qt}_{h}_{cc}")
                        pair_s = []
                        for ci in range(ncc):
                            c = 2 * cc + ci
                            s = max(0, 128 * (c - 4 * qt))
                            pair_s.append(s)
                            out = spsum[:, ci * 512 + s:(ci + 1) * 512]
                            hk = qk_sb[4 + h // 2][(h % 2) * 64:(h % 2) * 64 + 64,
                                                   c * 128:(c + 1) * 128]
                            nc.tensor.matmul(out, hk, hq[:, s:512],
                                             start=True, stop=True)
                        starts += pair_s
                        pt = ppool.tile([128, 1024], bf16, tag="p",
                                        name=f"p{qt}_{h}_{cc}")
                        s0 = pair_s[0]
                        if ncc == 2 and pair_s[1] > 0:
                            # diagonal pair: exact-range exps (skips the
                            # never-read gap columns on the bottleneck engine
                            # and lets the last PV chunk gate on a short op)
                            nc.scalar.activation(pt[:, s0:512], spsum[:, s0:512],
                                                 EXP, bias=0.0, scale=1.0)
                            s1 = pair_s[1]
                            nc.scalar.activation(pt[:, 512 + s1:1024],
                                                 spsum[:, 512 + s1:1024],
                                                 EXP, bias=0.0, scale=1.0)
                        else:
                            hi = 1024 if ncc == 2 else 512
                            nc.scalar.activation(pt[:, s0:hi], spsum[:, s0:hi],
                                                 EXP, bias=0.0, scale=1.0)
                        for ci in range(ncc):
                            c = 2 * cc + ci
                            if c >= 4 * qt:
                                s = pair_s[ci]
                                sl = pt[:, ci * 512 + s:ci * 512 + s + 128]
                                nc.vector.tensor_mul(sl, tri_sb, sl)
                        pts.append(pt)
                        slot += 1
                        if slot in pops and fillers[qt]:
                            fillers[qt].pop(0)()
                    pend.append((h, pts, starts, o_dense, qt, nk))
                    if len(pend) > 1:
                        emit_pv_tail(*pend.pop(0))

                if qt == QT_TILES - 1:
                    # the run's final PV has no S-phase behind it to cover
                    # its exp wait -- spend the held-back y tiles here, ahead
                    # of it in the queue, so the PE isn't idle while the last
                    # head's exp drains
                    for fn in late_y[:4]:
                        fn()
                for it in pend:
                    emit_pv_tail(*it)
                pend = []
                if qt == QT_TILES - 1:
                    # ...and the rest during the final softmax chain's
                    # DVE/DMA/gpsimd stages, after the last PV
                    for fn in late_y[4:]:
                        fn()
                    late_y = []
                if qt < QT_TILES - 1:
                    # hold 4 of qt2's y tiles back from the braid: emitted
                    # right after the final PV, they execute on the PE during
                    # the last softmax chain's DVE/DMA/gpsimd stages, filling
                    # what is otherwise pure PE idle before the y3 flush
                    for i, (j, ts) in enumerate(
                            (j, ts) for j in range(2) for ts in range(4)):
                        fn = (lambda qt=qt, j=j, ts=ts, oden=oden:
                              emit_y_tile(qt, j, ts, oden))
                        if qt == 2:
                            late_y.append(fn)
                        else:
                            (fillers[1] if qt == 0 else fillers[3]).append(fn)
                else:
                    for j in range(2):
                        for ts in range(4):
                            emit_y_tile(qt, j, ts, oden)

    nc.compile()
    return nc


_NC_CACHE = {}


def _get_nc(nkc=KC):
    if nkc not in _NC_CACHE:
        _NC_CACHE[nkc] = _build_nc(nkc)
    return _NC_CACHE[nkc]


def make_core_inputs(input, mask, w_qkv, b_qkv, w_out, b_out, core):
    """Host-side sharding/layout prep for one core."""
    b, g = core // 2, core % 2
    scale = 1.0 / np.sqrt(HD)

    import ml_dtypes
    xT = input[b].T                                                   # [H, T]
    # xp[p, tt*4096 + hc*512 + t] = xT[hc*128 + p, tt*512 + t]
    xp = np.ascontiguousarray(
        xT.reshape(8, 128, 4, 512).transpose(1, 2, 0, 3).reshape(128, 16384)
    ).astype(ml_dtypes.bfloat16)

    qcols = slice(g * GD, (g + 1) * GD)
    kcols = slice(H + g * GD, H + (g + 1) * GD)
    vcols = slice(2 * H + g * GD, 2 * H + (g + 1) * GD)
    wq = w_qkv[:, qcols] * scale
    wk = w_qkv[:, kcols]
    wqk = np.concatenate([wq, wk], axis=1)                            # [H, 2GD]
    # ct-major pack: wqkp[p, ct*1024 + hc*128 + m] = wqk[hc*128 + p, ct*128 + m]
    wqkp = np.ascontiguousarray(
        wqk.reshape(8, 128, 8, 128).transpose(1, 2, 0, 3).reshape(128, 8192)
    ).astype(ml_dtypes.bfloat16)
    bqk = np.concatenate([b_qkv[qcols] * scale, b_qkv[kcols]]).astype(np.float32)
    bqkc = np.ascontiguousarray(bqk.reshape(8, 128).T)               # [128, 8]
    wv = w_qkv[:, vcols]
    wvp = np.ascontiguousarray(
        wv.reshape(8, 128, 512).transpose(1, 0, 2).reshape(128, 4096)
    ).astype(ml_dtypes.bfloat16)
    bv = b_qkv[vcols][None, :].astype(np.float32)

    wout = w_out[g * GD:(g + 1) * GD, :]
    woutp = np.ascontiguousarray(
        wout.reshape(4, 128, 1024).transpose(1, 0, 2).reshape(128, 4096)
    ).astype(ml_dtypes.bfloat16)
    # b_out on core with g==0 only; zeros on g==1 (partials are summed on host)
    bout = (b_out if g == 0 else np.zeros_like(b_out))[None, :].astype(np.float32)

    padb01 = mask[b].astype(np.float32)                                # [T]
    padb01 = np.ascontiguousarray(padb01.reshape(KC, 128).T)           # [128, KC]
    pbq = np.concatenate([padb01, bqkc], axis=1).astype(np.float32)    # [128, 24]

    # single 128x128 upper-tri (col >= row) causal mask for diagonal blocks
    rr = np.arange(128)[:, None]
    cc = np.arange(128)[None, :]
    tri = np.where(cc >= rr, 1.0, 0.0).astype(ml_dtypes.bfloat16)

    return {
        "xp": xp, "wqkp": wqkp, "wvp": wvp, "pbq": pbq, "bv": bv,
        "woutp": woutp, "bout": bout, "tri": tri,
    }


def kernel(input, mask, w_qkv, b_qkv, w_out, b_out):
    from concourse.bass_utils import run_bass_kernel_spmd

    input = np.asarray(input)
    mask = np.asarray(mask)
    w_qkv = np.asarray(w_qkv)
    b_qkv = np.asarray(b_qkv)
    w_out = np.asarray(w_out)
    b_out = np.asarray(b_out)
    # all-padded key chunks (beyond every batch's valid length) are skipped
    nkc = int(min(KC, max(1, np.ceil(mask.sum(axis=1).max() / 128))))
    nc = _get_nc(nkc)
    in_maps = [
        make_core_inputs(input, mask, w_qkv, b_qkv, w_out, b_out, c)
        for c in range(NCORES)
    ]
    res = run_bass_kernel_spmd(nc, in_maps, list(range(NCORES)))
    parts = [np.asarray(res.results[c]["y"]).astype(np.float32)
             for c in range(NCORES)]
    out = np.stack([parts[2 * b] + parts[2 * b + 1] for b in range(B)])
    return out.astype(np.float32)


if __name__ == "__main__":
    nc = _build_nc()
    print("build ok")


# revision 61
# speedup vs baseline: 1.0015x; 1.0015x over previous
"""Causal multi-head attention (B=4, T=2048, H=1024, 16 heads) on 8 trn2 cores.

Sharding: batch(4) x head-group(2).  Core c -> batch b=c//2, heads g=c%2
(8 heads each): zero-communication data/tensor parallelism.  Each core
computes its QKV projection slice, causal+padding-masked attention for its 8
heads, and a row-parallel slice of the output projection; the two partial
outputs per batch row are summed on the host (row-parallel unshard).

Device algorithm (per core; attention kept transposed so softmax reduces
along the PE contraction dim, all matmuls at 1 cycle/row):
  DMA issue costs ~650ns each on the sync queue, so inputs arrive as a few
  large host-packed contiguous transfers in priority order: the first QK
  psum needs only ~1.25 MB (wqk ct0 + x column-block 0).
  ScalarE's exp stream is the phase-2 bottleneck (~171us irreducible at
  1 col/cycle), so projections are BRAIDED INTO the attention stream: a
  prologue emits only QK t-tile 0 and V chunks 0..3; QK tile tt and V
  chunks for q-tile qt+1 (plus the previous q-tile's output-projection
  tiles) are spread as PE filler across q-tile qt's S-pair slots, starting
  exp ~40us after launch and keeping the PE busy where exp gates PV.
  All-padded key chunks beyond ceil(max_len/128) (derived from the mask at
  runtime; program compiled+cached per value) are skipped entirely in
  S/exp/PV/V, and the K projection stops at that column.
  QT/KT [512, T] bf16 = wqk^T-slices @ xT  (Q pre-scaled by 1/sqrt(hd) on
      host; bias added on DVE during the PSUM->SBUF move)
  V [T, 8x65] bf16 = xT^T @ wv, bias via DVE add of a gpsimd-broadcast
      replicated row; a ones column per head; all 65 columns multiplied by
      the key-padding 0/1 mask (per-partition scalar) -> padded keys drop
      out of both the attention numerator and the softmax denominator.
  per (head, q-tile 512, k-chunk pair 2x128), exact-triangle streaming:
      S^T[k, q] = KT_h[:, chunk].T @ QT_h[:, qtile]     (bf16, f32 PSUM);
      diagonal chunks stream only q >= 128*c (partial-range matmul)
      P^T = exp(S^T)  (ScalarE, ONE op per pair over [s_even:1024) -- the
      never-read gap columns of diagonal pairs hold stale-but-finite psum)
      causal masking only on the 128x128 diagonal block of diagonal chunks:
      P^T *= upper-tri 0/1 mask on DVE (post-exp, off the ScalarE path)
      o^T[65, q] += V_aug[chunk, head].T @ P^T          (row 64 = denom;
      diagonal chunks accumulate only their valid q sub-range)
  softmax tail: copy denom row, DMA-shift to partition 0, approx-recip,
  gpsimd partition-broadcast, scale on DVE.  (partition_broadcast reads
  PHYSICAL partition 0 on hw regardless of the AP base, and the custom-DVE
  recip cannot read PSUM -- both were tried and produce garbage.)
  Odd head of each pair runs FIRST so the pair's last writer is the even
  head's direct [0:64) write -- the final y tiles never wait on a DMA shift.
  y[t, j] = sum_hp o_dense_hp[:, t].T @ wout_hp[:, j] (bf16) + b_out on DVE,
  stored bf16 (host upcasts and sums the two per-batch partials in f32)
"""

import os
import sys

import numpy as np

sys.path.insert(0, "/opt/trn_rl_repo")

B, T, H = 4, 2048, 1024
NH, HD = 16, 64
NCORES = 8
HPC = 8          # heads per core
GD = HPC * HD    # head dims per core = 512
KC = T // 128    # 16 k-chunks
QT_TILES = T // 512  # 4 q-tiles
HC = H // 128    # 8 h-chunks (contraction for projections)


def _build_nc(nkc=KC):
    import concourse.bass as bass
    import concourse.tile as tile
    import concourse.mybir as mybir
    from concourse import bacc
    from contextlib import ExitStack

    f32 = mybir.dt.float32
    bf16 = mybir.dt.bfloat16
    EXP = mybir.ActivationFunctionType.Exp

    nc = bacc.Bacc("TRN2", target_bir_lowering=False, debug=False)

    xp_d = nc.dram_tensor("xp", [128, 4 * 4096], bf16, kind="ExternalInput").ap()
    wqkp_d = nc.dram_tensor("wqkp", [128, 8 * 1024], bf16, kind="ExternalInput").ap()
    wvp_d = nc.dram_tensor("wvp", [128, 8 * 512], bf16, kind="ExternalInput").ap()
    pbq_d = nc.dram_tensor("pbq", [128, KC + 8], f32, kind="ExternalInput").ap()
    bv_d = nc.dram_tensor("bv", [1, GD], f32, kind="ExternalInput").ap()
    woutp_d = nc.dram_tensor("woutp", [128, 4 * H], bf16, kind="ExternalInput").ap()
    bout_d = nc.dram_tensor("bout", [1, H], f32, kind="ExternalInput").ap()
    tri_d = nc.dram_tensor("tri", [128, 128], bf16, kind="ExternalInput").ap()
    y_d = nc.dram_tensor("y", [T, H], bf16, kind="ExternalOutput").ap()

    def emit_v_proj(nc, bvrep, xtb, wv_sb, psv, v_sb, padb01_sb, ts, HC, HPC):
        tt, off = ts // 4, (ts % 4) * 128
        for hc in range(HC):
            nc.tensor.matmul(
                psv, xtb[tt][:, hc * 512 + off:hc * 512 + off + 128],
                wv_sb[:, hc * 512:(hc + 1) * 512],
                start=(hc == 0), stop=(hc == HC - 1))
        pad_c = padb01_sb[:, ts:ts + 1]
        dst = v_sb[ts].rearrange("p (h c) -> p h c", h=HPC)[:, :, 0:64]
        bsrc = bvrep.rearrange("p (h c) -> p h c", h=HPC)
        srcv = psv.rearrange("p (h c) -> p h c", h=HPC)
        nc.vector.tensor_add(dst, bsrc, srcv)
        nc.vector.tensor_scalar_mul(dst, dst, pad_c)
        onescols = v_sb[ts].rearrange("p (h c) -> p h c", h=HPC)[:, :, 64:65]
        nc.vector.memset(onescols, 1.0)
        nc.vector.tensor_scalar_mul(onescols, onescols, pad_c)

    with ExitStack() as ctx:
        tc = ctx.enter_context(tile.TileContext(nc))

        # Persistent activations
        acts = ctx.enter_context(tc.tile_pool(name="acts", bufs=1))
        qk_sb = [acts.tile([128, T], bf16, name=f"qk{i}") for i in range(8)]
        v_sb = [acts.tile([128, HPC * 65], bf16, name=f"v{c}") for c in range(KC)]

        # Inputs, few big DMAs in priority order: pbq, wqk(ct0), x(tt0),
        # wqk(ct1-7), x(tt1-3), wv, bv.
        p1c = ctx.enter_context(tc.tile_pool(name="p1c", bufs=1))
        pbq_sb = p1c.tile([128, KC + 8], f32, name="pbq_sb")
        padb01_sb = pbq_sb[:, 0:KC]
        bqkc_sb = pbq_sb[:, KC:KC + 8]
        nc.sync.dma_start(pbq_sb, pbq_d)

        xt_pool = ctx.enter_context(tc.tile_pool(name="xt", bufs=1))
        xtb = [xt_pool.tile([128, 8 * 512], bf16, name=f"xt{t}") for t in range(4)]
        wqk_pool = ctx.enter_context(tc.tile_pool(name="wqkp", bufs=1))
        wqkp_sb = wqk_pool.tile([128, 8 * 1024], bf16, name="wqkp_sb")
        wv_pool = ctx.enter_context(tc.tile_pool(name="wvp", bufs=1))
        wv_sb = wv_pool.tile([128, 8 * 512], bf16, name="wv_sb")

        nc.sync.dma_start(wqkp_sb[:, 0:1024], wqkp_d[:, 0:1024])
        nc.sync.dma_start(xtb[0][:, 0:2048], xp_d[:, 0:2048])
        nc.sync.dma_start(xtb[0][:, 2048:4096], xp_d[:, 2048:4096])
        nc.sync.dma_start(wqkp_sb[:, 1024:4096], wqkp_d[:, 1024:4096])
        nc.sync.dma_start(wqkp_sb[:, 4096:8192], wqkp_d[:, 4096:8192])
        nc.sync.dma_start(wv_sb, wvp_d)
        bv_sb = p1c.tile([1, GD], f32, name="bv_sb")
        nc.sync.dma_start(bv_sb, bv_d)
        tri_sb = p1c.tile([128, 128], bf16, name="tri_sb")
        nc.sync.dma_start(tri_sb, tri_d)
        for tt in range(1, 4):
            nc.sync.dma_start(xtb[tt], xp_d[:, tt * 4096:(tt + 1) * 4096])
        bvrep = p1c.tile([128, GD], f32, name="bvrep")
        nc.gpsimd.partition_broadcast(bvrep, bv_sb)

        # PE p-state warm-up staging: borrow qk_sb[0] (overwritten later by
        # the real projection) as a memset-zeroed matmul operand so the
        # warm-up has no DMA dependency.
        warm_sb = qk_sb[0][:, 0:640]
        nc.vector.memset(warm_sb, 0.0)

        # ------------- single phase: projections braided into attention ----
        with ExitStack() as p2:
            p2c = p2.enter_context(tc.tile_pool(name="p2c", bufs=1))
            bout_sb = p2c.tile([1, H], f32, name="bout_sb")
            nc.sync.dma_start(bout_sb, bout_d)
            brep = p2c.tile([128, H], f32, name="brep")
            nc.gpsimd.partition_broadcast(brep, bout_sb)
            wout_sb = p2c.tile([128, 4 * H], bf16, name="wo")
            nc.sync.dma_start(wout_sb, woutp_d)

            ppool = p2.enter_context(tc.tile_pool(name="pchunks", bufs=16))
            osc_pool = p2.enter_context(tc.tile_pool(name="osc", bufs=4))
            oden_pool = p2.enter_context(tc.tile_pool(name="oden", bufs=12))
            dpool = p2.enter_context(tc.tile_pool(name="dtiles", bufs=4))
            ypool = p2.enter_context(tc.tile_pool(name="ysb", bufs=3))
            ps_s = p2.enter_context(tc.tile_pool(name="ps_s", bufs=2, space="PSUM"))
            ps_o = p2.enter_context(tc.tile_pool(name="ps_o", bufs=2, space="PSUM"))
            ps_y = p2.enter_context(tc.tile_pool(name="ps_y", bufs=2, space="PSUM"))

            def emit_qk_ct(tt, ct):
                """one Q^T/K^T projection column-tile (K clamped to nkc keys)"""
                w = min(512, max(0, nkc * 128 - tt * 512)) if ct >= 4 else 512
                if w == 0:
                    return
                ps = ps_y.tile([128, 512], f32, tag="y", name=f"psqk{ct}_{tt}")
                for hc in range(HC):
                    nc.tensor.matmul(
                        ps[:, 0:w],
                        wqkp_sb[:, ct * 1024 + hc * 128:ct * 1024 + (hc + 1) * 128],
                        xtb[tt][:, hc * 512:hc * 512 + w],
                        start=(hc == 0), stop=(hc == HC - 1))
                nc.vector.tensor_scalar_add(
                    qk_sb[ct][:, tt * 512:tt * 512 + w], ps[:, 0:w],
                    bqkc_sb[:, ct:ct + 1])

            warm_ps = ps_y.tile([128, 512], f32, tag="y", name="warm_ps")
            for _ in range(16):
                nc.tensor.matmul(warm_ps, warm_sb[:, 0:128], warm_sb[:, 128:640],
                                 start=True, stop=True)

            # Prologue: only the QK tiles and V chunks q-tile 0 needs.  The
            # remaining projection tiles are braided into the attention
            # stream as PE filler -- ScalarE's exp stream (the phase-2
            # bottleneck) starts ~45us earlier this way.
            for ct in range(8):
                emit_qk_ct(0, ct)
            for ts in range(4):
                emit_v_proj(nc, bvrep, xtb, wv_sb,
                            ps_y.tile([128, 512], f32, tag="y", name=f"psv{ts}"),
                            v_sb, padb01_sb, ts, HC, HPC)

            def attn_tail(qt, h, opsum, o_dense):
                """softmax denom -> recip (straight off psum p64) -> DMA shift
                to p0 -> gpsimd broadcast -> scale -> dense repack"""
                stage = dpool.tile([65, 512], f32, tag="dstage", name=f"st{qt}_{h}")
                nc.vector.tensor_copy(stage[64:65, :], opsum[64:65, :])
                dp0 = dpool.tile([1, 512], f32, tag="dp0", name=f"dp0_{qt}_{h}")
                nc.sync.dma_start(dp0, stage[64:65, :])
                rp0 = dpool.tile([1, 512], f32, tag="rp0", name=f"rp0_{qt}_{h}")
                nc.vector.reciprocal_approx_fast(rp0, dp0)
                rrep = dpool.tile([64, 512], f32, tag="rrep", name=f"rr{qt}_{h}")
                nc.gpsimd.partition_broadcast(rrep, rp0)
                if h % 2 == 0:
                    nc.vector.tensor_mul(o_dense[0:64, :], rrep, opsum[0:64, :])
                else:
                    o_sc = osc_pool.tile([64, 512], bf16, tag="osc", name=f"osc{qt}_{h}")
                    nc.vector.tensor_mul(o_sc, rrep, opsum[0:64, :])
                    nc.sync.dma_start(o_dense[64:128, :], o_sc)

            def emit_y_tile(qt, j, ts, oden):
                """one output-projection tile for q-tile qt (b_out via preload)"""
                q0 = qt * 512
                ypsum = ps_y.tile([128, 512], f32, tag="y", name=f"y{qt}_{j}_{ts}")
                for hp in range(4):
                    nc.tensor.matmul(
                        ypsum,
                        oden[hp][:, ts * 128:(ts + 1) * 128],
                        wout_sb[:, hp * 1024 + j * 512:hp * 1024 + (j + 1) * 512],
                        start=(hp == 0), stop=(hp == 3))
                ysb = ypool.tile([128, 512], bf16, tag="ysb", name=f"ys{qt}_{j}_{ts}")
                nc.vector.tensor_add(ysb, brep[:, j * 512:(j + 1) * 512], ypsum)
                nc.sync.dma_start(
                    y_d[q0 + ts * 128:q0 + (ts + 1) * 128, j * 512:(j + 1) * 512],
                    ysb)

            # Just-in-time braided PE filler, spread evenly over each q-tile's
            # pair slots.  QK projection tile tt (needed first by q-tile tt)
            # and V chunks arrive one q-tile before first use; y tiles of
            # q-tile 0 braid into qt1, y of 1 and 2 into qt3 (the most
            # exp-bound stretch), y of 3 flush at the end.
            fillers = {qt: [] for qt in range(QT_TILES)}
            for vqt in range(QT_TILES - 1):
                for ct in range(8):
                    fillers[vqt].append(
                        lambda tt=vqt + 1, ct=ct: emit_qk_ct(tt, ct))
                for c in range(4 * (vqt + 1), min(4 * (vqt + 2), nkc)):
                    fillers[vqt].append(
                        lambda c=c: emit_v_proj(
                            nc, bvrep, xtb, wv_sb,
                            ps_y.tile([128, 512], f32, tag="y", name=f"psvd{c}"),
                            v_sb, padb01_sb, c, HC, HPC))

            def emit_pv_tail(h, pts, starts, o_dense, pqt, pnk):
                opsum = ps_o.tile([65, 512], f32, tag="o", name=f"o{pqt}_{h}")
                for c in range(pnk):
                    s = starts[c]
                    nc.tensor.matmul(
                        opsum[:, s:512],
                        v_sb[c][:, h * 65:(h + 1) * 65].bitcast(bf16),
                        pts[c // 2][:, (c % 2) * 512 + s:(c % 2) * 512 + 512],
                        start=(c == 0), stop=(c == pnk - 1),
                        skip_group_check=True)
                attn_tail(pqt, h, opsum, o_dense)

            # one-head software-pipeline skew, carried ACROSS q-tile
            # boundaries: S(next head) always runs between S(h) and PV(h)
            # so exp(h) has a full extra S-phase to drain before PV fires
            pend = []
            late_y = []
            for qt in range(QT_TILES):
                q0 = qt * 512
                nk = min(4 * (qt + 1), nkc)
                nslots = ((nk + 1) // 2) * HPC
                nfill = len(fillers[qt])
                pops = {round((i + 1) * nslots / (nfill + 1)) for i in range(nfill)}
                slot = 0
                oden = []
                for h in [1, 0, 3, 2, 5, 4, 7, 6]:
                    if len(oden) <= h // 2:
                        o_dense = oden_pool.tile([128, 512], bf16, tag="od",
                                                 name=f"od{qt}_{h // 2}")
                        oden.append(o_dense)
                    else:
                        o_dense = oden[h // 2]
                    hq = qk_sb[h // 2][(h % 2) * 64:(h % 2) * 64 + 64, q0:q0 + 512]
                    # S^T in two-chunk psum tiles, whole-head S stream first
                    # (exp trails on ScalarE), then the dense PV stream.
                    # Diagonal chunks stream/accumulate only q >= 128*c.
                    pts = []
                    starts = []
                    for cc in range((nk + 1) // 2):
                        ncc = min(2, nk - 2 * cc)   # chunks in this pair
                        spsum = ps_s.tile([128, 1024], f32, tag="s",
                                          name=f"s{qt}_{h}_{cc}")
                        pair_s = []
                        for ci in range(ncc):
                            c = 2 * cc + ci
                            s = max(0, 128 * (c - 4 * qt))
                            pair_s.append(s)
                            out = spsum[:, ci * 512 + s:(ci + 1) * 512]
                            hk = qk_sb[4 + h // 2][(h % 2) * 64:(h % 2) * 64 + 64,
                                                   c * 128:(c + 1) * 128]
                            nc.tensor.matmul(out, hk, hq[:, s:512],
                                             start=True, stop=True)
                        starts += pair_s
                        pt = ppool.tile([128, 1024], bf16, tag="p",
                                        name=f"p{qt}_{h}_{cc}")
                        s0 = pair_s[0]
                        if ncc == 2 and pair_s[1] > 0:
                            # diagonal pair: exact-range exps (skips the
                            # never-read gap columns on the bottleneck engine
                            # and lets the last PV chunk gate on a short op)
                            nc.scalar.activation(pt[:, s0:512], spsum[:, s0:512],
                                                 EXP, bias=0.0, scale=1.0)
                            s1 = pair_s[1]
                            nc.scalar.activation(pt[:, 512 + s1:1024],
                                                 spsum[:, 512 + s1:1024],
                                                 EXP, bias=0.0, scale=1.0)
                        else:
                            hi = 1024 if ncc == 2 else 512
                            nc.scalar.activation(pt[:, s0:hi], spsum[:, s0:hi],
                                                 EXP, bias=0.0, scale=1.0)
                        for ci in range(ncc):
                            c = 2 * cc + ci
                            if c >= 4 * qt:
                                s = pair_s[ci]
                                sl = pt[:, ci * 512 + s:ci * 512 + s + 128]
                                nc.vector.tensor_mul(sl, tri_sb, sl)
                        pts.append(pt)
                        slot += 1
                        if slot in pops and fillers[qt]:
                            fillers[qt].pop(0)()
                    pend.append((h, pts, starts, o_dense, qt, nk))
                    if len(pend) > 1:
                        emit_pv_tail(*pend.pop(0))

                if qt == QT_TILES - 1:
                    # the run's final PV has no S-phase behind it to cover
                    # its exp wait -- spend the held-back y tiles here, ahead
                    # of it in the queue, so the PE isn't idle while the last
                    # head's exp drains
                    for fn in late_y[:4]:
                        fn()
                for it in pend:
                    emit_pv_tail(*it)
                pend = []
                if qt == QT_TILES - 1:
                    # ...and the rest during the final softmax chain's
                    # DVE/DMA/gpsimd stages, after the last PV
                    for fn in late_y[4:]:
                        fn()
                    late_y = []
                if qt < QT_TILES - 1:
                    # hold 4 of qt2's y tiles back from the braid: emitted
                    # right after the final PV, they execute on the PE during
                    # the last softmax chain's DVE/DMA/gpsimd stages, filling
                    # what is otherwise pure PE idle before the y3 flush
                    for i, (j, ts) in enumerate(
                            (j, ts) for j in range(2) for ts in range(4)):
                        fn = (lambda qt=qt, j=j, ts=ts, oden=oden:
                              emit_y_tile(qt, j, ts, oden))
                        if qt == 2:
                            late_y.append(fn)
                        else:
                            (fillers[1] if qt == 0 else fillers[3]).append(fn)
                else:
                    for j in range(2):
                        for ts in range(4):
                            emit_y_tile(qt, j, ts, oden)

    nc.compile()
    return nc


_NC_CACHE = {}


def _get_nc(nkc=KC):
    if nkc not in _NC_CACHE:
        _NC_CACHE[nkc] = _build_nc(nkc)
    return _NC_CACHE[nkc]


def make_core_inputs(input, mask, w_qkv, b_qkv, w_out, b_out, core):
    """Host-side sharding/layout prep for one core."""
    b, g = core // 2, core % 2
    scale = 1.0 / np.sqrt(HD)

    import ml_dtypes
    xT = input[b].T                                                   # [H, T]
    # xp[p, tt*4096 + hc*512 + t] = xT[hc*128 + p, tt*512 + t]
    xp = np.ascontiguousarray(
        xT.reshape(8, 128, 4, 512).transpose(1, 2, 0, 3).reshape(128, 16384)
    ).astype(ml_dtypes.bfloat16)

    qcols = slice(g * GD, (g + 1) * GD)
    kcols = slice(H + g * GD, H + (g + 1) * GD)
    vcols = slice(2 * H + g * GD, 2 * H + (g + 1) * GD)
    wq = w_qkv[:, qcols] * scale
    wk = w_qkv[:, kcols]
    wqk = np.concatenate([wq, wk], axis=1)                            # [H, 2GD]
    # ct-major pack: wqkp[p, ct*1024 + hc*128 + m] = wqk[hc*128 + p, ct*128 + m]
    wqkp = np.ascontiguousarray(
        wqk.reshape(8, 128, 8, 128).transpose(1, 2, 0, 3).reshape(128, 8192)
    ).astype(ml_dtypes.bfloat16)
    bqk = np.concatenate([b_qkv[qcols] * scale, b_qkv[kcols]]).astype(np.float32)
    bqkc = np.ascontiguousarray(bqk.reshape(8, 128).T)               # [128, 8]
    wv = w_qkv[:, vcols]
    wvp = np.ascontiguousarray(
        wv.reshape(8, 128, 512).transpose(1, 0, 2).reshape(128, 4096)
    ).astype(ml_dtypes.bfloat16)
    bv = b_qkv[vcols][None, :].astype(np.float32)

    wout = w_out[g * GD:(g + 1) * GD, :]
    woutp = np.ascontiguousarray(
        wout.reshape(4, 128, 1024).transpose(1, 0, 2).reshape(128, 4096)
    ).astype(ml_dtypes.bfloat16)
    # b_out on core with g==0 only; zeros on g==1 (partials are summed on host)
    bout = (b_out if g == 0 else np.zeros_like(b_out))[None, :].astype(np.float32)

    padb01 = mask[b].astype(np.float32)                                # [T]
    padb01 = np.ascontiguousarray(padb01.reshape(KC, 128).T)           # [128, KC]
    pbq = np.concatenate([padb01, bqkc], axis=1).astype(np.float32)    # [128, 24]

    # single 128x128 upper-tri (col >= row) causal mask for diagonal blocks
    rr = np.arange(128)[:, None]
    cc = np.arange(128)[None, :]
    tri = np.where(cc >= rr, 1.0, 0.0).astype(ml_dtypes.bfloat16)

    return {
        "xp": xp, "wqkp": wqkp, "wvp": wvp, "pbq": pbq, "bv": bv,
        "woutp": woutp, "bout": bout, "tri": tri,
    }


def kernel(input, mask, w_qkv, b_qkv, w_out, b_out):
    from concourse.bass_utils import run_bass_kernel_spmd

    input = np.asarray(input)
    mask = np.asarray(mask)
    w_qkv = np.asarray(w_qkv)
    b_qkv = np.asarray(b_qkv)
    w_out = np.asarray(w_out)
    b_out = np.asarray(b_out)
    # all-padded key chunks (beyond every batch's valid length) are skipped
    nkc = int(min(KC, max(1, np.ceil(mask.sum(axis=1).max() / 128))))
    nc = _get_nc(nkc)
    in_maps = [
        make_core_inputs(input, mask, w_qkv, b_qkv, w_out, b_out, c)
        for c in range(NCORES)
    ]
    res = run_bass_kernel_spmd(nc, in_maps, list(range(NCORES)))
    parts = [np.asarray(res.results[c]["y"]).astype(np.float32)
             for c in range(NCORES)]
    out = np.stack([parts[2 * b] + parts[2 * b + 1] for b in range(B)])
    return out.astype(np.float32)


if __name__ == "__main__":
    nc = _build_nc()
    print("build ok")


# revision 62
# speedup vs baseline: 1.0057x; 1.0042x over previous
"""Causal multi-head attention (B=4, T=2048, H=1024, 16 heads) on 8 trn2 cores.

Sharding: batch(4) x head-group(2).  Core c -> batch b=c//2, heads g=c%2
(8 heads each): zero-communication data/tensor parallelism.  Each core
computes its QKV projection slice, causal+padding-masked attention for its 8
heads, and a row-parallel slice of the output projection; the two partial
outputs per batch row are summed on the host (row-parallel unshard).

Device algorithm (per core; attention kept transposed so softmax reduces
along the PE contraction dim, all matmuls at 1 cycle/row):
  DMA issue costs ~650ns each on the sync queue, so inputs arrive as a few
  large host-packed contiguous transfers in priority order: the first QK
  psum needs only ~1.25 MB (wqk ct0 + x column-block 0).
  ScalarE's exp stream is the phase-2 bottleneck (~171us irreducible at
  1 col/cycle), so projections are BRAIDED INTO the attention stream: a
  prologue emits only QK t-tile 0 and V chunks 0..3; QK tile tt and V
  chunks for q-tile qt+1 (plus the previous q-tile's output-projection
  tiles) are spread as PE filler across q-tile qt's S-pair slots, starting
  exp ~40us after launch and keeping the PE busy where exp gates PV.
  All-padded key chunks beyond ceil(max_len/128) (derived from the mask at
  runtime; program compiled+cached per value) are skipped entirely in
  S/exp/PV/V, and the K projection stops at that column.
  QT/KT [512, T] bf16 = wqk^T-slices @ xT  (Q pre-scaled by 1/sqrt(hd) on
      host; bias added on DVE during the PSUM->SBUF move)
  V [T, 8x65] bf16 = xT^T @ wv, bias via DVE add of a gpsimd-broadcast
      replicated row; a ones column per head; all 65 columns multiplied by
      the key-padding 0/1 mask (per-partition scalar) -> padded keys drop
      out of both the attention numerator and the softmax denominator.
  per (head, q-tile 512, k-chunk pair 2x128), exact-triangle streaming:
      S^T[k, q] = KT_h[:, chunk].T @ QT_h[:, qtile]     (bf16, f32 PSUM);
      diagonal chunks stream only q >= 128*c (partial-range matmul)
      P^T = exp(S^T)  (ScalarE, ONE op per pair over [s_even:1024) -- the
      never-read gap columns of diagonal pairs hold stale-but-finite psum)
      causal masking only on the 128x128 diagonal block of diagonal chunks:
      P^T *= upper-tri 0/1 mask on DVE (post-exp, off the ScalarE path)
      o^T[65, q] += V_aug[chunk, head].T @ P^T          (row 64 = denom;
      diagonal chunks accumulate only their valid q sub-range)
  softmax tail: copy denom row, DMA-shift to partition 0, approx-recip,
  gpsimd partition-broadcast, scale on DVE.  (partition_broadcast reads
  PHYSICAL partition 0 on hw regardless of the AP base, and the custom-DVE
  recip cannot read PSUM -- both were tried and produce garbage.)
  Odd head of each pair runs FIRST so the pair's last writer is the even
  head's direct [0:64) write -- the final y tiles never wait on a DMA shift.
  y[t, j] = sum_hp o_dense_hp[:, t].T @ wout_hp[:, j] (bf16) + b_out on DVE,
  stored bf16 (host upcasts and sums the two per-batch partials in f32)
"""

import os
import sys

import numpy as np

sys.path.insert(0, "/opt/trn_rl_repo")

B, T, H = 4, 2048, 1024
NH, HD = 16, 64
NCORES = 8
HPC = 8          # heads per core
GD = HPC * HD    # head dims per core = 512
KC = T // 128    # 16 k-chunks
QT_TILES = T // 512  # 4 q-tiles
HC = H // 128    # 8 h-chunks (contraction for projections)


def _build_nc(nkc=KC):
    import concourse.bass as bass
    import concourse.tile as tile
    import concourse.mybir as mybir
    from concourse import bacc
    from contextlib import ExitStack

    f32 = mybir.dt.float32
    bf16 = mybir.dt.bfloat16
    EXP = mybir.ActivationFunctionType.Exp

    nc = bacc.Bacc("TRN2", target_bir_lowering=False, debug=False)

    xp_d = nc.dram_tensor("xp", [128, 4 * 4096], bf16, kind="ExternalInput").ap()
    wqkp_d = nc.dram_tensor("wqkp", [128, 8 * 1024], bf16, kind="ExternalInput").ap()
    wvp_d = nc.dram_tensor("wvp", [128, 8 * 512], bf16, kind="ExternalInput").ap()
    pbq_d = nc.dram_tensor("pbq", [128, KC + 8], f32, kind="ExternalInput").ap()
    bv_d = nc.dram_tensor("bv", [1, GD], f32, kind="ExternalInput").ap()
    woutp_d = nc.dram_tensor("woutp", [128, 4 * H], bf16, kind="ExternalInput").ap()
    bout_d = nc.dram_tensor("bout", [1, H], f32, kind="ExternalInput").ap()
    tri_d = nc.dram_tensor("tri", [128, 128], bf16, kind="ExternalInput").ap()
    y_d = nc.dram_tensor("y", [T, H], bf16, kind="ExternalOutput").ap()

    def emit_v_proj(nc, bvrep, xtb, wv_sb, psv, v_sb, padb01_sb, ts, HC, HPC):
        tt, off = ts // 4, (ts % 4) * 128
        for hc in range(HC):
            nc.tensor.matmul(
                psv, xtb[tt][:, hc * 512 + off:hc * 512 + off + 128],
                wv_sb[:, hc * 512:(hc + 1) * 512],
                start=(hc == 0), stop=(hc == HC - 1))
        pad_c = padb01_sb[:, ts:ts + 1]
        dst = v_sb[ts].rearrange("p (h c) -> p h c", h=HPC)[:, :, 0:64]
        bsrc = bvrep.rearrange("p (h c) -> p h c", h=HPC)
        srcv = psv.rearrange("p (h c) -> p h c", h=HPC)
        nc.vector.tensor_add(dst, bsrc, srcv)
        nc.vector.tensor_scalar_mul(dst, dst, pad_c)
        onescols = v_sb[ts].rearrange("p (h c) -> p h c", h=HPC)[:, :, 64:65]
        nc.vector.memset(onescols, 1.0)
        nc.vector.tensor_scalar_mul(onescols, onescols, pad_c)

    with ExitStack() as ctx:
        tc = ctx.enter_context(tile.TileContext(nc))

        # Persistent activations
        acts = ctx.enter_context(tc.tile_pool(name="acts", bufs=1))
        qk_sb = [acts.tile([128, T], bf16, name=f"qk{i}") for i in range(8)]
        v_sb = [acts.tile([128, HPC * 65], bf16, name=f"v{c}") for c in range(KC)]

        # Inputs, few big DMAs in priority order: pbq, wqk(ct0), x(tt0),
        # wqk(ct1-7), x(tt1-3), wv, bv.
        p1c = ctx.enter_context(tc.tile_pool(name="p1c", bufs=1))
        pbq_sb = p1c.tile([128, KC + 8], f32, name="pbq_sb")
        padb01_sb = pbq_sb[:, 0:KC]
        bqkc_sb = pbq_sb[:, KC:KC + 8]
        nc.sync.dma_start(pbq_sb, pbq_d)

        xt_pool = ctx.enter_context(tc.tile_pool(name="xt", bufs=1))
        xtb = [xt_pool.tile([128, 8 * 512], bf16, name=f"xt{t}") for t in range(4)]
        wqk_pool = ctx.enter_context(tc.tile_pool(name="wqkp", bufs=1))
        wqkp_sb = wqk_pool.tile([128, 8 * 1024], bf16, name="wqkp_sb")
        wv_pool = ctx.enter_context(tc.tile_pool(name="wvp", bufs=1))
        wv_sb = wv_pool.tile([128, 8 * 512], bf16, name="wv_sb")

        nc.sync.dma_start(wqkp_sb[:, 0:1024], wqkp_d[:, 0:1024])
        nc.sync.dma_start(xtb[0][:, 0:2048], xp_d[:, 0:2048])
        nc.sync.dma_start(xtb[0][:, 2048:4096], xp_d[:, 2048:4096])
        nc.sync.dma_start(wqkp_sb[:, 1024:4096], wqkp_d[:, 1024:4096])
        nc.sync.dma_start(wqkp_sb[:, 4096:8192], wqkp_d[:, 4096:8192])
        nc.sync.dma_start(wv_sb, wvp_d)
        bv_sb = p1c.tile([1, GD], f32, name="bv_sb")
        nc.sync.dma_start(bv_sb, bv_d)
        tri_sb = p1c.tile([128, 128], bf16, name="tri_sb")
        nc.sync.dma_start(tri_sb, tri_d)
        for tt in range(1, 4):
            nc.sync.dma_start(xtb[tt], xp_d[:, tt * 4096:(tt + 1) * 4096])
        bvrep = p1c.tile([128, GD], f32, name="bvrep")
        nc.gpsimd.partition_broadcast(bvrep, bv_sb)

        # PE p-state warm-up staging: borrow qk_sb[0] (overwritten later by
        # the real projection) as a memset-zeroed matmul operand so the
        # warm-up has no DMA dependency.
        warm_sb = qk_sb[0][:, 0:640]
        nc.vector.memset(warm_sb, 0.0)

        # ------------- single phase: projections braided into attention ----
        with ExitStack() as p2:
            p2c = p2.enter_context(tc.tile_pool(name="p2c", bufs=1))
            bout_sb = p2c.tile([1, H], f32, name="bout_sb")
            nc.sync.dma_start(bout_sb, bout_d)
            brep = p2c.tile([128, H], f32, name="brep")
            nc.gpsimd.partition_broadcast(brep, bout_sb)
            wout_sb = p2c.tile([128, 4 * H], bf16, name="wo")
            nc.sync.dma_start(wout_sb, woutp_d)

            ppool = p2.enter_context(tc.tile_pool(name="pchunks", bufs=16))
            osc_pool = p2.enter_context(tc.tile_pool(name="osc", bufs=4))
            oden_pool = p2.enter_context(tc.tile_pool(name="oden", bufs=12))
            dpool = p2.enter_context(tc.tile_pool(name="dtiles", bufs=4))
            ypool = p2.enter_context(tc.tile_pool(name="ysb", bufs=3))
            ps_s = p2.enter_context(tc.tile_pool(name="ps_s", bufs=2, space="PSUM"))
            ps_o = p2.enter_context(tc.tile_pool(name="ps_o", bufs=2, space="PSUM"))
            ps_y = p2.enter_context(tc.tile_pool(name="ps_y", bufs=2, space="PSUM"))

            def emit_qk_ct(tt, ct):
                """one Q^T/K^T projection column-tile (K clamped to nkc keys)"""
                w = min(512, max(0, nkc * 128 - tt * 512)) if ct >= 4 else 512
                if w == 0:
                    return
                ps = ps_y.tile([128, 512], f32, tag="y", name=f"psqk{ct}_{tt}")
                for hc in range(HC):
                    nc.tensor.matmul(
                        ps[:, 0:w],
                        wqkp_sb[:, ct * 1024 + hc * 128:ct * 1024 + (hc + 1) * 128],
                        xtb[tt][:, hc * 512:hc * 512 + w],
                        start=(hc == 0), stop=(hc == HC - 1))
                nc.vector.tensor_scalar_add(
                    qk_sb[ct][:, tt * 512:tt * 512 + w], ps[:, 0:w],
                    bqkc_sb[:, ct:ct + 1])

            warm_ps = ps_s.tile([128, 1024], f32, tag="s", name="warm_ps")
            for _ in range(24):
                nc.tensor.matmul(warm_ps[:, 0:512], warm_sb[:, 0:128],
                                 warm_sb[:, 128:640], start=True, stop=True)

            # Prologue: only the QK tiles and V chunks q-tile 0 needs.  The
            # remaining projection tiles are braided into the attention
            # stream as PE filler -- ScalarE's exp stream (the phase-2
            # bottleneck) starts ~45us earlier this way.
            for ct in range(8):
                emit_qk_ct(0, ct)
            for ts in range(4):
                emit_v_proj(nc, bvrep, xtb, wv_sb,
                            ps_y.tile([128, 512], f32, tag="y", name=f"psv{ts}"),
                            v_sb, padb01_sb, ts, HC, HPC)

            def attn_tail(qt, h, opsum, o_dense):
                """softmax denom -> recip (straight off psum p64) -> DMA shift
                to p0 -> gpsimd broadcast -> scale -> dense repack"""
                stage = dpool.tile([65, 512], f32, tag="dstage", name=f"st{qt}_{h}")
                nc.vector.tensor_copy(stage[64:65, :], opsum[64:65, :])
                dp0 = dpool.tile([1, 512], f32, tag="dp0", name=f"dp0_{qt}_{h}")
                nc.sync.dma_start(dp0, stage[64:65, :])
                rp0 = dpool.tile([1, 512], f32, tag="rp0", name=f"rp0_{qt}_{h}")
                nc.vector.reciprocal_approx_fast(rp0, dp0)
                rrep = dpool.tile([64, 512], f32, tag="rrep", name=f"rr{qt}_{h}")
                nc.gpsimd.partition_broadcast(rrep, rp0)
                if h % 2 == 0:
                    nc.vector.tensor_mul(o_dense[0:64, :], rrep, opsum[0:64, :])
                else:
                    o_sc = osc_pool.tile([64, 512], bf16, tag="osc", name=f"osc{qt}_{h}")
                    nc.vector.tensor_mul(o_sc, rrep, opsum[0:64, :])
                    nc.sync.dma_start(o_dense[64:128, :], o_sc)

            def emit_y_tile(qt, j, ts, oden):
                """one output-projection tile for q-tile qt (b_out via preload)"""
                q0 = qt * 512
                ypsum = ps_y.tile([128, 512], f32, tag="y", name=f"y{qt}_{j}_{ts}")
                for hp in range(4):
                    nc.tensor.matmul(
                        ypsum,
                        oden[hp][:, ts * 128:(ts + 1) * 128],
                        wout_sb[:, hp * 1024 + j * 512:hp * 1024 + (j + 1) * 512],
                        start=(hp == 0), stop=(hp == 3))
                ysb = ypool.tile([128, 512], bf16, tag="ysb", name=f"ys{qt}_{j}_{ts}")
                nc.vector.tensor_add(ysb, brep[:, j * 512:(j + 1) * 512], ypsum)
                nc.sync.dma_start(
                    y_d[q0 + ts * 128:q0 + (ts + 1) * 128, j * 512:(j + 1) * 512],
                    ysb)

            # Just-in-time braided PE filler, spread evenly over each q-tile's
            # pair slots.  QK projection tile tt (needed first by q-tile tt)
            # and V chunks arrive one q-tile before first use; y tiles of
            # q-tile 0 braid into qt1, y of 1 and 2 into qt3 (the most
            # exp-bound stretch), y of 3 flush at the end.
            fillers = {qt: [] for qt in range(QT_TILES)}
            for vqt in range(QT_TILES - 1):
                for ct in range(8):
                    fillers[vqt].append(
                        lambda tt=vqt + 1, ct=ct: emit_qk_ct(tt, ct))
                for c in range(4 * (vqt + 1), min(4 * (vqt + 2), nkc)):
                    fillers[vqt].append(
                        lambda c=c: emit_v_proj(
                            nc, bvrep, xtb, wv_sb,
                            ps_y.tile([128, 512], f32, tag="y", name=f"psvd{c}"),
                            v_sb, padb01_sb, c, HC, HPC))

            def emit_pv_tail(h, pts, starts, o_dense, pqt, pnk):
                opsum = ps_o.tile([65, 512], f32, tag="o", name=f"o{pqt}_{h}")
                for c in range(pnk):
                    s = starts[c]
                    nc.tensor.matmul(
                        opsum[:, s:512],
                        v_sb[c][:, h * 65:(h + 1) * 65].bitcast(bf16),
                        pts[c // 2][:, (c % 2) * 512 + s:(c % 2) * 512 + 512],
                        start=(c == 0), stop=(c == pnk - 1),
                        skip_group_check=True)
                attn_tail(pqt, h, opsum, o_dense)

            # one-head software-pipeline skew, carried ACROSS q-tile
            # boundaries: S(next head) always runs between S(h) and PV(h)
            # so exp(h) has a full extra S-phase to drain before PV fires
            pend = []
            late_y = []
            for qt in range(QT_TILES):
                q0 = qt * 512
                nk = min(4 * (qt + 1), nkc)
                nslots = ((nk + 1) // 2) * HPC
                nfill = len(fillers[qt])
                pops = {round((i + 1) * nslots / (nfill + 1)) for i in range(nfill)}
                slot = 0
                oden = []
                for h in [1, 0, 3, 2, 5, 4, 7, 6]:
                    if len(oden) <= h // 2:
                        o_dense = oden_pool.tile([128, 512], bf16, tag="od",
                                                 name=f"od{qt}_{h // 2}")
                        oden.append(o_dense)
                    else:
                        o_dense = oden[h // 2]
                    hq = qk_sb[h // 2][(h % 2) * 64:(h % 2) * 64 + 64, q0:q0 + 512]
                    # S^T in two-chunk psum tiles, whole-head S stream first
                    # (exp trails on ScalarE), then the dense PV stream.
                    # Diagonal chunks stream/accumulate only q >= 128*c.
                    pts = []
                    starts = []
                    for cc in range((nk + 1) // 2):
                        ncc = min(2, nk - 2 * cc)   # chunks in this pair
                        spsum = ps_s.tile([128, 1024], f32, tag="s",
                                          name=f"s{qt}_{h}_{cc}")
                        pair_s = []
                        for ci in range(ncc):
                            c = 2 * cc + ci
                            s = max(0, 128 * (c - 4 * qt))
                            pair_s.append(s)
                            out = spsum[:, ci * 512 + s:(ci + 1) * 512]
                            hk = qk_sb[4 + h // 2][(h % 2) * 64:(h % 2) * 64 + 64,
                                                   c * 128:(c + 1) * 128]
                            nc.tensor.matmul(out, hk, hq[:, s:512],
                                             start=True, stop=True)
                        starts += pair_s
                        pt = ppool.tile([128, 1024], bf16, tag="p",
                                        name=f"p{qt}_{h}_{cc}")
                        s0 = pair_s[0]
                        if ncc == 2 and pair_s[1] > 0:
                            # diagonal pair: exact-range exps (skips the
                            # never-read gap columns on the bottleneck engine
                            # and lets the last PV chunk gate on a short op)
                            nc.scalar.activation(pt[:, s0:512], spsum[:, s0:512],
                                                 EXP, bias=0.0, scale=1.0)
                            s1 = pair_s[1]
                            nc.scalar.activation(pt[:, 512 + s1:1024],
                                                 spsum[:, 512 + s1:1024],
                                                 EXP, bias=0.0, scale=1.0)
                        else:
                            hi = 1024 if ncc == 2 else 512
                            nc.scalar.activation(pt[:, s0:hi], spsum[:, s0:hi],
                                                 EXP, bias=0.0, scale=1.0)
                        for ci in range(ncc):
                            c = 2 * cc + ci
                            if c >= 4 * qt:
                                s = pair_s[ci]
                                sl = pt[:, ci * 512 + s:ci * 512 + s + 128]
                                nc.vector.tensor_mul(sl, tri_sb, sl)
                        pts.append(pt)
                        slot += 1
                        if slot in pops and fillers[qt]:
                            fillers[qt].pop(0)()
                    pend.append((h, pts, starts, o_dense, qt, nk))
                    if len(pend) > 1:
                        emit_pv_tail(*pend.pop(0))

                if qt == QT_TILES - 1:
                    # the run's final PV has no S-phase behind it to cover
                    # its exp wait -- spend the held-back y tiles here, ahead
                    # of it in the queue, so the PE isn't idle while the last
                    # head's exp drains
                    for fn in late_y[:4]:
                        fn()
                for it in pend:
                    emit_pv_tail(*it)
                pend = []
                if qt == QT_TILES - 1:
                    # ...and the rest during the final softmax chain's
                    # DVE/DMA/gpsimd stages, after the last PV
                    for fn in late_y[4:]:
                        fn()
                    late_y = []
                if qt < QT_TILES - 1:
                    # hold 4 of qt2's y tiles back from the braid: emitted
                    # right after the final PV, they execute on the PE during
                    # the last softmax chain's DVE/DMA/gpsimd stages, filling
                    # what is otherwise pure PE idle before the y3 flush
                    for i, (j, ts) in enumerate(
                            (j, ts) for j in range(2) for ts in range(4)):
                        fn = (lambda qt=qt, j=j, ts=ts, oden=oden:
                              emit_y_tile(qt, j, ts, oden))
                        if qt == 2:
                            late_y.append(fn)
                        else:
                            (fillers[1] if qt == 0 else fillers[3]).append(fn)
                else:
                    for j in range(2):
                        for ts in range(4):
                            emit_y_tile(qt, j, ts, oden)

    nc.compile()
    return nc


_NC_CACHE = {}


def _get_nc(nkc=KC):
    if nkc not in _NC_CACHE:
        _NC_CACHE[nkc] = _build_nc(nkc)
    return _NC_CACHE[nkc]


def make_core_inputs(input, mask, w_qkv, b_qkv, w_out, b_out, core):
    """Host-side sharding/layout prep for one core."""
    b, g = core // 2, core % 2
    scale = 1.0 / np.sqrt(HD)

    import ml_dtypes
    xT = input[b].T                                                   # [H, T]
    # xp[p, tt*4096 + hc*512 + t] = xT[hc*128 + p, tt*512 + t]
    xp = np.ascontiguousarray(
        xT.reshape(8, 128, 4, 512).transpose(1, 2, 0, 3).reshape(128, 16384)
    ).astype(ml_dtypes.bfloat16)

    qcols = slice(g * GD, (g + 1) * GD)
    kcols = slice(H + g * GD, H + (g + 1) * GD)
    vcols = slice(2 * H + g * GD, 2 * H + (g + 1) * GD)
    wq = w_qkv[:, qcols] * scale
    wk = w_qkv[:, kcols]
    wqk = np.concatenate([wq, wk], axis=1)                            # [H, 2GD]
    # ct-major pack: wqkp[p, ct*1024 + hc*128 + m] = wqk[hc*128 + p, ct*128 + m]
    wqkp = np.ascontiguousarray(
        wqk.reshape(8, 128, 8, 128).transpose(1, 2, 0, 3).reshape(128, 8192)
    ).astype(ml_dtypes.bfloat16)
    bqk = np.concatenate([b_qkv[qcols] * scale, b_qkv[kcols]]).astype(np.float32)
    bqkc = np.ascontiguousarray(bqk.reshape(8, 128).T)               # [128, 8]
    wv = w_qkv[:, vcols]
    wvp = np.ascontiguousarray(
        wv.reshape(8, 128, 512).transpose(1, 0, 2).reshape(128, 4096)
    ).astype(ml_dtypes.bfloat16)
    bv = b_qkv[vcols][None, :].astype(np.float32)

    wout = w_out[g * GD:(g + 1) * GD, :]
    woutp = np.ascontiguousarray(
        wout.reshape(4, 128, 1024).transpose(1, 0, 2).reshape(128, 4096)
    ).astype(ml_dtypes.bfloat16)
    # b_out on core with g==0 only; zeros on g==1 (partials are summed on host)
    bout = (b_out if g == 0 else np.zeros_like(b_out))[None, :].astype(np.float32)

    padb01 = mask[b].astype(np.float32)                                # [T]
    padb01 = np.ascontiguousarray(padb01.reshape(KC, 128).T)           # [128, KC]
    pbq = np.concatenate([padb01, bqkc], axis=1).astype(np.float32)    # [128, 24]

    # single 128x128 upper-tri (col >= row) causal mask for diagonal blocks
    rr = np.arange(128)[:, None]
    cc = np.arange(128)[None, :]
    tri = np.where(cc >= rr, 1.0, 0.0).astype(ml_dtypes.bfloat16)

    return {
        "xp": xp, "wqkp": wqkp, "wvp": wvp, "pbq": pbq, "bv": bv,
        "woutp": woutp, "bout": bout, "tri": tri,
    }


def kernel(input, mask, w_qkv, b_qkv, w_out, b_out):
    from concourse.bass_utils import run_bass_kernel_spmd

    input = np.asarray(input)
    mask = np.asarray(mask)
    w_qkv = np.asarray(w_qkv)
    b_qkv = np.asarray(b_qkv)
    w_out = np.asarray(w_out)
    b_out = np.asarray(b_out)
    # all-padded key chunks (beyond every batch's valid length) are skipped
    nkc = int(min(KC, max(1, np.ceil(mask.sum(axis=1).max() / 128))))
    nc = _get_nc(nkc)
    in_maps = [
        make_core_inputs(input, mask, w_qkv, b_qkv, w_out, b_out, c)
        for c in range(NCORES)
    ]
    res = run_bass_kernel_spmd(nc, in_maps, list(range(NCORES)))
    parts = [np.asarray(res.results[c]["y"]).astype(np.float32)
             for c in range(NCORES)]
    out = np.stack([parts[2 * b] + parts[2 * b + 1] for b in range(B)])
    return out.astype(np.float32)


if __name__ == "__main__":
    nc = _build_nc()
    print("build ok")
